# revision 1
# baseline (speedup 1.0000x reference)
"""Trainium2 Bass kernel for nn_Net_71270687310327 (scatter_memory).

Computation (see reference):
  - keys = (timings+1)*512 + slot_index, with argmin(surprise*0.9) slot's key
    overridden to its slot index (forces rank 0, stable-sort tiebreak exact).
  - rank[b,m] = #{m' : key[b,m'] < key[b,m]}  (all keys distinct)
  - pred_in = [sorted memory rows | timing bits], fed to a 4-layer MLP.

Sharding: W0 row-sharded over 8 cores by slot-rank range (64 ranks/core,
17024 rows of W0 each, fully contiguous HBM reads). Each core gathers only
its 64 ranks' memory rows (dma_gather), computes a partial h = pred_in @ W0
contribution, AllReduce over the 8 cores, then every core redundantly runs
the small W1/W2/Wout layers.

The same program runs on all 8 cores (SPMD); all per-core differences are
carried by per-core input constants (W0 shard, rank-range constants).
"""

import sys, os

sys.path.insert(0, "/opt/trn_rl_repo")

import numpy as np

import concourse.bass as bass
import concourse.bacc as bacc
import concourse.mybir as mybir
from concourse import tile
from concourse import bass_utils

class _SkipRest(Exception):
    pass


F32 = mybir.dt.float32
I16 = mybir.dt.int16
ALU = mybir.AluOpType
ACTF = mybir.ActivationFunctionType

B, M, V, H, TD = 32, 512, 256, 1024, 10
NC = 8
RPC = M // NC            # 64 ranks per core
MEMROWS = B * M          # 16384
MEMP = MEMROWS + B       # 16416 (gather source rows: memory rows + x rows)
NKT = RPC * V // 128     # 128 mem k-tiles per core
NBT = RPC * TD // 128    # 5 bits k-tiles per core
W0S_ROWS = RPC * V + RPC * TD  # 17024
NIDX = RPC * B           # 2048 gather indices per core


def build_program(stage="full"):
    nkt_lim = NKT + NBT
    if stage.startswith("parth") and stage != "parth":
        nkt_lim = int(stage[5:])
        stage = "parth"
    lvl = {"idx": 0, "tk": 1, "parth": 2, "full": 3}[stage]
    nc = bacc.Bacc(
        "TRN2",
        target_bir_lowering=False,
        debug=False,
        enable_asserts=False,
        num_devices=NC,
    )

    def din(name, shape, dtype=F32):
        return nc.dram_tensor(name, list(shape), dtype, kind="ExternalInput").ap()

    mem_plus = din("mem_plus", (MEMP, V))
    timings = din("timings", (B, M))
    msur = din("msur", (B, M))
    w0s = din("W0s", (W0S_ROWS, H))
    w1 = din("W1", (H, H))
    w2 = din("W2", (H, H))
    wout = din("Wout", (H, V))
    b0r = din("b0r", (B, H))
    b1r = din("b1r", (B, H))
    b2r = din("b2r", (B, H))
    boutr = din("boutr", (B, V))
    c_eye = din("c_eye", (128, 128))
    c_esel = din("c_esel", (B, B * 128))
    c_iota = din("c_iota512", (B, M))
    c_iotam = din("c_iotam", (128, 4))
    c_rrow = din("c_rrow", (128, RPC))
    c_sel16 = din("c_sel16", (1, 16 * 128))
    c_amask = din("c_amask", (128, 128))
    c_coff = din("c_coff", (128, 128))
    c_rtd = din("c_rtd", (RPC, NBT * TD * 128))

    out = nc.dram_tensor("out", [B, V], F32, kind="ExternalOutput").ap()
    dbg = (nc.dram_tensor("dbg", [128, 256], F32, kind="ExternalOutput").ap()
           if stage != "full" else None)

    with tile.TileContext(nc) as tc:
        with (
            tc.tile_pool(name="const", bufs=1) as constp,
            tc.tile_pool(name="state", bufs=1) as state,
            tc.tile_pool(name="wres", bufs=1) as wres,
            tc.tile_pool(name="krep", bufs=2) as krepp,
            tc.tile_pool(name="pt", bufs=8) as ptp,
            tc.tile_pool(name="w0t", bufs=6) as w0p,
            tc.tile_pool(name="pk", bufs=1, space="PSUM") as pkp,
            tc.tile_pool(name="pflat", bufs=1, space="PSUM") as pflatp,
            tc.tile_pool(name="psort", bufs=1, space="PSUM") as psortp,
            tc.tile_pool(name="ptr", bufs=2, space="PSUM") as ptrp,
            tc.tile_pool(name="ph", bufs=1, space="PSUM") as php,
            tc.tile_pool(name="dram", bufs=1, space="DRAM") as dramp,
        ):
            # ---- constants / small state into SBUF
            def load(pool, ap):
                t = pool.tile(list(ap.shape), ap.dtype, tag=f"ld_{ap.tensor.name}")
                nc.sync.dma_start(t[:], ap)
                return t

            eye = load(constp, c_eye)
            esel = load(constp, c_esel)
            iota = load(constp, c_iota)
            iotam = load(constp, c_iotam)
            rrow = load(constp, c_rrow)
            sel16 = load(constp, c_sel16)
            amask = load(constp, c_amask)
            coff = load(constp, c_coff)
            rtd = load(constp, c_rtd)
            b0s = load(constp, b0r)
            b1s = load(constp, b1r)
            b2s = load(constp, b2r)
            bouts = load(constp, boutr)
            t_sb = load(state, timings)
            ms_sb = load(state, msur)

            # resident output-layer weights; W1/W2 stream through the k-tile pool
            wos = wres.tile([128, 8 * V], F32, tag="wos")
            for kt in range(8):
                nc.sync.dma_start(wos[:, kt * V:(kt + 1) * V], wout[kt * 128:(kt + 1) * 128, :])

            # ---- stage A: keys -------------------------------------------
            msur2 = state.tile([B, M], F32, tag="msur2")
            nc.vector.tensor_scalar(msur2[:], ms_sb[:], 0.9, None, ALU.mult)
            minv = state.tile([B, 1], F32, tag="minv")
            nc.vector.tensor_reduce(minv[:], msur2[:], axis=mybir.AxisListType.X, op=ALU.min)
            mask = state.tile([B, M], mybir.dt.uint8, tag="mask")
            nc.vector.tensor_scalar(mask[:], msur2[:], minv[:], None, ALU.is_equal)
            cand = state.tile([B, M], F32, tag="cand")
            nc.vector.memset(cand[:], 1.0e9)
            nc.vector.copy_predicated(cand[:], mask[:], iota[:])
            idx0 = state.tile([B, 1], F32, tag="idx0")
            nc.vector.tensor_reduce(idx0[:], cand[:], axis=mybir.AxisListType.X, op=ALU.min)

            keys = state.tile([B, M], F32, tag="keys")
            # (t+1)*512 + m  =  t*512 + 512 + m
            nc.vector.tensor_scalar(keys[:], t_sb[:], 512.0, 512.0, ALU.mult, ALU.add)
            nc.vector.tensor_tensor(keys[:], keys[:], iota[:], ALU.add)
            mask2 = state.tile([B, M], mybir.dt.uint8, tag="mask2")
            nc.vector.tensor_scalar(mask2[:], iota[:], idx0[:], None, ALU.is_equal)
            nc.vector.copy_predicated(keys[:], mask2[:], iota[:])

            # ---- keysT via PE transpose ----------------------------------
            keysT = state.tile([128, 4 * B], F32, tag="keysT")
            for mt in range(4):
                ptt = ptrp.tile([128, 128], F32, tag="pm")
                nc.tensor.transpose(ptt[:, 0:B], keys[:, mt * 128:(mt + 1) * 128], eye[0:B, 0:B])
                nc.scalar.activation(keysT[:, mt * B:(mt + 1) * B], ptt[:, 0:B], ACTF.Copy)

            # ---- ranks, P^T, order/sorted extraction ---------------------
            rank_sb = state.tile([128, 4 * B], F32, tag="rank")
            scratch = state.tile([128, M], F32, tag="scratch")
            flat = state.tile([1, NIDX], F32, tag="flat")
            psort_t = psortp.tile([RPC, B], F32, tag="psort")
            for g in range(4):
                pflat_t = pflatp.tile([1, 8 * RPC], F32, tag="pflat")
                for b8 in range(8):
                    b = g * 8 + b8
                    pk_t = pkp.tile([128, M], F32, tag="pkrep")
                    nc.tensor.matmul(pk_t[:], esel[:, b * 128:(b + 1) * 128], keys[:],
                                     start=True, stop=True)
                    krep = krepp.tile([128, M], F32, tag="krep")
                    nc.scalar.activation(krep[:], pk_t[:], ACTF.Copy)
                    for mt in range(4):
                        nc.vector.tensor_scalar(
                            scratch[:], krep[:], keysT[:, mt * B + b:mt * B + b + 1], None,
                            ALU.is_lt, ALU.add,
                            accum_out=rank_sb[:, b * 4 + mt:b * 4 + mt + 1])
                    pts = []
                    for mt in range(4):
                        pt_t = ptp.tile([128, RPC], F32, tag="pt")
                        nc.vector.tensor_scalar(
                            pt_t[:], rrow[:], rank_sb[:, b * 4 + mt:b * 4 + mt + 1], None,
                            ALU.is_equal)
                        pts.append(pt_t)
                    for mt in range(4):
                        nc.tensor.matmul(
                            pflat_t[0:1, b8 * RPC:(b8 + 1) * RPC],
                            iotam[:, mt:mt + 1], pts[mt][:],
                            start=(mt == 0), stop=(mt == 3))
                        nc.tensor.matmul(
                            psort_t[0:RPC, b:b + 1],
                            pts[mt][:], keysT[:, mt * B + b:mt * B + b + 1],
                            start=(mt == 0), stop=(mt == 3))
                nc.scalar.activation(flat[0:1, g * 512:(g + 1) * 512], pflat_t[:], ACTF.Copy)

            # ---- bits from sorted keys -----------------------------------
            # binary decomposition of sorted key (< 2^19); timing bit d of t
            # is key bit d+9.  u_all[:, d*B:(d+1)*B] = bit (d+9) of key.
            skT = state.tile([RPC, B], F32, tag="skT")
            nc.scalar.activation(skT[:], psort_t[:], ACTF.Copy)
            rem = state.tile([RPC, B], F32, tag="rem")
            nc.vector.tensor_copy(rem[:], skT[:])
            u_all = state.tile([RPC, TD * B], F32, tag="u_all")
            tmpu = state.tile([RPC, B], F32, tag="tmpu")
            for j in range(18, 8, -1):
                d = j - 9
                ud = u_all[:, d * B:(d + 1) * B]
                nc.vector.tensor_scalar(ud, rem[:], float(2 ** j), None, ALU.is_ge)
                nc.vector.tensor_scalar(tmpu[:], ud, float(2 ** j), None, ALU.mult)
                nc.vector.tensor_tensor(rem[:], rem[:], tmpu[:], ALU.subtract)
            # bits_sb[t][p, b] = u_{d(p)}[r(p), b] via selection matmuls
            bits_sb = state.tile([128, NBT * B], F32, tag="bits")
            for t in range(NBT):
                pb = ptrp.tile([128, 128], F32, tag="pm")
                for d in range(TD):
                    nc.tensor.matmul(pb[:, 0:B],
                                     rtd[:, (t * TD + d) * 128:(t * TD + d + 1) * 128],
                                     u_all[:, d * B:(d + 1) * B],
                                     start=(d == 0), stop=(d == TD - 1))
                nc.scalar.activation(bits_sb[:, t * B:(t + 1) * B], pb[:, 0:B], ACTF.Copy)

            # ---- gather indices ------------------------------------------
            pidx_t = ptrp.tile([128, 128], F32, tag="pm")
            flat_v = flat.rearrange("p (n s) -> p n s", s=16)
            for k in range(16):
                nc.tensor.matmul(pidx_t[:], sel16[0:1, k * 128:(k + 1) * 128],
                                 flat_v[:, :, k], start=(k == 0), stop=(k == 15))
            tmpidx = state.tile([128, 128], F32, tag="tmpidx")
            nc.vector.tensor_tensor(tmpidx[:], pidx_t[:], amask[:], ALU.mult)
            idx_sb = state.tile([128, 128], I16, tag="idx")
            nc.vector.tensor_tensor(idx_sb[:], tmpidx[:], coff[:], ALU.add)

            if stage == "idx":
                nc.vector.tensor_copy(tmpidx[:], idx_sb[:])
                nc.sync.dma_start(dbg[:, 0:128], tmpidx[:])
                nc.sync.dma_start(dbg[:, 128:256], bits_sb[:, 0:128])
            do_rest = lvl >= 1
            try:
              if not do_rest:
                  raise _SkipRest
              # ---- gather + transpose to pred_in^T tiles -------------------
              G = state.tile([128, 16 * V], F32, tag="G")
              nc.gpsimd.dma_gather(
                  out_ap=G.rearrange("p (c e) -> p c e", e=V),
                  in_ap=mem_plus,
                  idxs_ap=idx_sb[:],
                  num_idxs=NIDX,
                  num_idxs_reg=NIDX,
                  elem_size=V,
                  single_packet=False,
              )
              T_all = state.tile([128, 16 * V], F32, tag="T_all")
              for c in range(16):
                  for hh in range(2):
                      off = c * V + hh * 128
                      pt2 = ptrp.tile([128, 128], F32, tag="pm")
                      nc.tensor.transpose(pt2[:], G[:, off:off + 128], eye[:])
                      nc.scalar.activation(T_all[:, off:off + 128], pt2[:], ACTF.Copy)

              # ---- repack transposed tiles to k-tile-major contiguous ------
              # T_all col = 256*cb + 128*h + 64*b2 + r  ->  TK col = 64*r + 32*h + 2*cb + b2
              TK = state.tile([128, 16 * V], F32, tag="TK")
              t_in = T_all.rearrange("p (cb h b2 r) -> p r h cb b2", cb=16, h=2, b2=2, r=RPC)
              tk_out = TK.rearrange("p (r h cb b2) -> p r h cb b2", r=RPC, h=2, cb=16, b2=2)
              nc.vector.tensor_copy(tk_out[:], t_in[:])

              if stage == "tk":
                  nc.sync.dma_start(dbg[:, 0:256], TK[:, 0:256])
              if lvl < 2:
                  raise _SkipRest
              # ---- main matmul: partial h = pred_in_shard @ W0_shard -------
              ph_t = php.tile([B, H], F32, tag="ph")
              for kt in range(nkt_lim):
                  w0t = w0p.tile([128, H], F32, tag="w0t")
                  nc.sync.dma_start(w0t[:], w0s[kt * 128:(kt + 1) * 128, :])
                  if kt < NKT:
                      lhsT = TK[:, kt * B:(kt + 1) * B]
                  else:
                      tb = kt - NKT
                      lhsT = bits_sb[:, tb * B:(tb + 1) * B]
                  last = kt == nkt_lim - 1
                  nc.tensor.matmul(ph_t[:, 0:512], lhsT, w0t[:, 0:512],
                                   start=(kt == 0), stop=last)
                  nc.tensor.matmul(ph_t[:, 512:1024], lhsT, w0t[:, 512:1024],
                                   start=(kt == 0), stop=last)

              # ---- AllReduce partial h over the 8 cores --------------------
              part_h = state.tile([B, H], F32, tag="part_h")
              nc.vector.tensor_copy(part_h[:], ph_t[:])
              if stage == "parth":
                  nc.sync.dma_start(dbg[0:B, 0:256], part_h[:, 0:256])
              if lvl < 3:
                  raise _SkipRest
              cc_in = dramp.tile([B, H], F32, tag="cc_in")
              cc_out = dramp.tile([B, H], F32, tag="cc_out")
              nc.sync.dma_start(cc_in[:], part_h[:])
              nc.gpsimd.collective_compute(
                  "AllReduce", ALU.add,
                  replica_groups=[list(range(NC))],
                  ins=[cc_in.opt()],
                  outs=[cc_out.opt()],
              )
              h_sb = state.tile([B, H], F32, tag="h_sb")
              nc.sync.dma_start(h_sb[:], cc_out[:])

              # ---- dense layers (replicated on every core) -----------------
              nc.vector.tensor_tensor(h_sb[:], h_sb[:], b0s[:], ALU.add)
              nc.vector.tensor_scalar(h_sb[:], h_sb[:], 0.0, None, ALU.max)

              def dense(h_in, w_dram, w_sb, bias_sb, n_out, relu, tag):
                  hT = state.tile([128, 8 * B], F32, tag=f"hT_{tag}")
                  for kt in range(8):
                      ptt = ptrp.tile([128, 128], F32, tag="pm")
                      nc.tensor.transpose(ptt[:, 0:B], h_in[:, kt * 128:(kt + 1) * 128], eye[0:B, 0:B])
                      nc.scalar.activation(hT[:, kt * B:(kt + 1) * B], ptt[:, 0:B], ACTF.Copy)
                  pho = php.tile([B, n_out], F32, tag="ph")
                  for kt in range(8):
                      if w_dram is not None:
                          wt = w0p.tile([128, H], F32, tag="w0t")
                          nc.sync.dma_start(wt[:, 0:n_out], w_dram[kt * 128:(kt + 1) * 128, :])
                      else:
                          wt = None
                      for j0 in range(0, n_out, 512):
                          jn = min(512, n_out - j0)
                          rhs = (wt[:, j0:j0 + jn] if wt is not None
                                 else w_sb[:, kt * n_out + j0:kt * n_out + j0 + jn])
                          nc.tensor.matmul(
                              pho[:, j0:j0 + jn], hT[:, kt * B:(kt + 1) * B], rhs,
                              start=(kt == 0), stop=(kt == 7))
                  h_next = state.tile([B, n_out], F32, tag=f"h_{tag}")
                  nc.vector.tensor_tensor(h_next[:], pho[:], bias_sb[:], ALU.add)
                  if relu:
                      nc.vector.tensor_scalar(h_next[:], h_next[:], 0.0, None, ALU.max)
                  return h_next

              h1 = dense(h_sb, w1, None, b1s, H, True, "l1")
              h2 = dense(h1, w2, None, b2s, H, True, "l2")
              logits = dense(h2, None, wos, bouts, V, False, "lo")
              nc.sync.dma_start(out, logits[:])
            except _SkipRest:
                pass

    nc.compile()
    return nc


def make_in_maps(inputs):
    x = np.asarray(inputs["x"], np.float32)
    memory = np.asarray(inputs["memory"], np.float32)
    timings = np.asarray(inputs["memory_timings"], np.float32)
    msur = np.asarray(inputs["memory_surprise"], np.float32)
    W0 = np.asarray(inputs["W0"], np.float32)
    W1 = np.asarray(inputs["W1"], np.float32)
    W2 = np.asarray(inputs["W2"], np.float32)
    Wout = np.asarray(inputs["Wout"], np.float32)
    b0 = np.asarray(inputs["b0"], np.float32)
    b1 = np.asarray(inputs["b1"], np.float32)
    b2 = np.asarray(inputs["b2"], np.float32)
    bout = np.asarray(inputs["bout"], np.float32)

    mem_plus = np.concatenate([memory.reshape(MEMROWS, V), x], 0)

    # shared constants
    eye = np.eye(128, dtype=np.float32)
    esel = np.zeros((B, B * 128), np.float32)
    for b in range(B):
        esel[b, b * 128:(b + 1) * 128] = 1.0
    iota512 = np.broadcast_to(np.arange(M, dtype=np.float32), (B, M)).copy()
    iotam = np.empty((128, 4), np.float32)
    for mt in range(4):
        iotam[:, mt] = np.arange(128) + mt * 128
    sel16 = np.zeros((1, 16 * 128), np.float32)
    for k in range(16):
        p = np.arange(128)
        sel16[0, k * 128:(k + 1) * 128] = (p % 16 == k)
    rtd = np.zeros((RPC, NBT * TD * 128), np.float32)
    for t in range(NBT):
        for p in range(128):
            l = t * 128 + p
            rp, dp = l // TD, l % TD
            rtd[rp, (t * TD + dp) * 128 + p] = 1.0

    shared = {
        "mem_plus": mem_plus,
        "timings": timings,
        "msur": msur,
        "W1": W1, "W2": W2, "Wout": Wout,
        "b0r": np.broadcast_to(b0, (B, H)).copy(),
        "b1r": np.broadcast_to(b1, (B, H)).copy(),
        "b2r": np.broadcast_to(b2, (B, H)).copy(),
        "boutr": np.broadcast_to(bout, (B, V)).copy(),
        "c_eye": eye, "c_esel": esel, "c_iota512": iota512,
        "c_iotam": iotam, "c_sel16": sel16, "c_rtd": rtd,
    }

    in_maps = []
    p = np.arange(128)
    f = np.arange(128)
    ii = 16 * f[None, :] + (p % 16)[:, None]   # [128,128] linear gather positions
    bb = ii // RPC
    rr = ii % RPC
    for core in range(NC):
        w0shard = np.concatenate(
            [W0[core * RPC * V:(core + 1) * RPC * V],
             W0[M * V + core * RPC * TD: M * V + (core + 1) * RPC * TD]], 0)
        rrowc = np.broadcast_to(
            np.arange(core * RPC, (core + 1) * RPC, dtype=np.float32), (128, RPC)).copy()
        am = np.ones((128, 128), np.float32)
        co = (512.0 * bb).astype(np.float32)
        if core == 0:
            r0 = rr == 0
            am[r0] = 0.0
            co[r0] = (MEMROWS + bb)[r0]
        m = dict(shared)
        m["W0s"] = np.ascontiguousarray(w0shard)
        m["c_rrow"] = rrowc
        m["c_amask"] = am
        m["c_coff"] = co
        in_maps.append(m)
    return in_maps


_NC_CACHE = None


def kernel(**inputs) -> np.ndarray:
    global _NC_CACHE
    if _NC_CACHE is None:
        _NC_CACHE = build_program()
    nc = _NC_CACHE
    in_maps = make_in_maps(inputs)
    res = bass_utils.run_bass_kernel_spmd(nc, in_maps, core_ids=list(range(NC)))
    return np.asarray(res.results[0]["out"], np.float32)


if __name__ == "__main__":
    np.random.seed(0)
    build_program()
    print("build OK")

